# revision 18
# baseline (speedup 1.0000x reference)
"""Causal single-head attention (B=4, T=4096, C=1024, H=64) on 8 TRN2 cores.

Sharding: core = 2*b + h (b = batch, h = kv-parity). Each core computes, for
ALL queries of its batch, partial softmax numerator/denominator over the kv
chunks (128 rows) of parity h; the host combines the two cores of a batch:
    out = (num0 + num1) / (den0 + den1).

Query columns are permuted parity-major within each 512-query superblock
(chunk order [h, 2+h, 1-h, 3-h]) so each superblock's parity-h kv columns sit
at positions 0:256 of both half-supers -- K/V projections read the resident
x tiles directly and the program is identical across cores (SPMD); host-built
masks and host-side row unpermutation absorb the data dependence.

Precision/engine plan (per core):
  - x is shipped twice: fp8(e4m3) full [C,T] for Q/K projections, and bf16
    parity-half [C,T/2] for the V projection (fp8 v noise is too large for
    early rows; bf16 x keeps v accurate).
  - Q/K projections run as fp8 DoubleRow matmuls (2 contraction rows/cycle)
    producing qT/kT directly in the [32, 2(hj), 512] layout that the
    DoubleRow S matmul consumes. Weights are pre-scaled by 64 so fp8 sees
    well-ranged values; the exp scale absorbs the 64*64.
  - S^T = k.q per kv chunk: fp8 DoubleRow, out [128kv, 512q] f32 PSUM.
  - P = exp(S*scale) on ACT -> fp8 pt [128, 2, 512]; diagonal pair masked by
    a host-built full mask (tri + dead + parity data-dependence).
  - PV: fp8 DoubleRow over kv pairs: lhsT = vaug [64, 2, 65] halves
    (v columns + ones column for den), accumulating outT [65, 512] f32.
  - Attention super 0 (queries 0:512) runs its P*V in bf16 (pt bf16 +
    bf16 vaug) and is emitted LAST (shortest drain tail) -- few-kv softmax
    rows are too sensitive to fp8 noise.
  - outT copied to bf16 SBUF (DVE) and DMA'd as [65, 512]; host transposes/
    unpermutes/combines.
  - Input DMA issues ride the (otherwise idle) GPSIMD/Pool engine via SWDGE,
    split so the kv-parity half (th=0) of each x pair lands first (K-proj can
    start before the full pair arrives); masks multiply on Pool too. All
    PSUM-reading casts/copies are on DVE.
"""

import numpy as np
import ml_dtypes

import concourse.bass as bass
import concourse.bacc as bacc
import concourse.tile as tile
from concourse import mybir
from concourse.bass_utils import run_bass_kernel_spmd

F32 = mybir.dt.float32
BF16 = mybir.dt.bfloat16
FP8 = mybir.dt.float8e4
DR = mybir.MatmulPerfMode.DoubleRow

E4NP = ml_dtypes.float8_e4m3
BFNP = ml_dtypes.bfloat16

B = 4
C = 1024
H = 64
SUP = 512
KC = 128


def chunk_perm(h):
    return [h, 2 + h, 1 - h, 3 - h]


DEFAULT_ORDER = ("k0 q1 k1 q2 v0 p1 q3 v1 p2 k2 q4 p3 q5 v2 p4 "
                 "k3 q6 p5 q7 v3 p6 q0 p7 p0").split()


def build_nc(T=4096, reps=1, order=None, dma_eng="gpsimd", mask_eng="gpsimd",
             memset_eng="gpsimd"):
    if order is None:
        order = DEFAULT_ORDER
    n_sup = T // SUP            # 8 attention supers
    n_pair = n_sup // 2         # 4 x-pairs == 4 parity kv supers
    scale = float(C) ** -0.5 / 4096.0

    nc = bacc.Bacc(None, target_bir_lowering=False)
    # [p, pr, jc, i, cc2, t]
    xq8_d = nc.dram_tensor("xq8", [128, n_pair, 2, 2, 4, SUP], FP8,
                           kind="ExternalInput")
    xvb_d = nc.dram_tensor("xvb", [128, n_pair, 8, SUP], BF16,
                           kind="ExternalInput")
    wq8_d = nc.dram_tensor("wq8", [128, 4, 2, 2, 32], FP8, kind="ExternalInput")
    wk8_d = nc.dram_tensor("wk8", [128, 4, 2, 2, 32], FP8, kind="ExternalInput")
    wvb_d = nc.dram_tensor("wvb", [128, 8, H], BF16, kind="ExternalInput")
    bias_d = nc.dram_tensor("bias", [128, 4], F32, kind="ExternalInput")
    mask8_d = nc.dram_tensor("mask8", [128, 2, SUP], FP8, kind="ExternalInput")
    maskb_d = nc.dram_tensor("maskb", [128, 2, SUP], BF16, kind="ExternalInput")
    idb_d = nc.dram_tensor("idb", [128, 128], BF16, kind="ExternalInput")
    out_d = nc.dram_tensor("out", [reps, n_sup, H + 1, SUP], BF16,
                           kind="ExternalOutput")

    with tile.TileContext(nc) as tc:
        with (
            tc.tile_pool(name="consts", bufs=1) as consts,
            tc.tile_pool(name="x8p", bufs=1) as x8p,
            tc.tile_pool(name="xvp", bufs=1) as xvp,
            tc.tile_pool(name="kTp", bufs=2) as kTp,
            tc.tile_pool(name="qTp", bufs=1) as qTp,
            tc.tile_pool(name="vTp", bufs=1) as vTp,
            tc.tile_pool(name="vap", bufs=1) as vap,
            tc.tile_pool(name="pt", bufs=16) as ptp,
            tc.tile_pool(name="ot", bufs=3) as otsb,
            tc.tile_pool(name="proj", bufs=1, space="PSUM") as projp,
            tc.tile_pool(name="spsum", bufs=2, space="PSUM") as spp,
            tc.tile_pool(name="otp", bufs=1, space="PSUM") as otp,
            tc.tile_pool(name="vxp", bufs=1, space="PSUM") as vxp,
        ):
            # ---- consts (small, SP HWDGE) ----
            wq8_sb = consts.tile([128, 4, 2, 2, 32], FP8)
            nc.sync.dma_start(wq8_sb[:], wq8_d[:])
            wk8_sb = consts.tile([128, 4, 2, 2, 32], FP8)
            nc.sync.dma_start(wk8_sb[:], wk8_d[:])
            wvb_sb = consts.tile([128, 8, H], BF16)
            nc.sync.dma_start(wvb_sb[:], wvb_d[:])
            bias_sb = consts.tile([128, 4], F32)
            nc.sync.dma_start(bias_sb[:], bias_d[:])
            mask8_sb = consts.tile([128, 2, SUP], FP8)
            nc.sync.dma_start(mask8_sb[:], mask8_d[:])
            maskb_sb = consts.tile([128, 2, SUP], BF16)
            nc.sync.dma_start(maskb_sb[:], maskb_d[:])
            idb_sb = consts.tile([128, 128], BF16)
            nc.sync.dma_start(idb_sb[:], idb_d[:])

            for rep in range(reps):
                x8t = {}
                xvt = {}

                def load_x8(pr):
                    t_ = x8p.tile([128, 2, 2, 4, SUP], FP8, tag=f"x8_{pr}")
                    getattr(nc, dma_eng).dma_start(t_[:], xq8_d[:, pr])
                    x8t[pr] = t_

                def load_xv(s):
                    t_ = xvp.tile([128, 8, SUP], BF16, tag=f"xv_{s}")
                    getattr(nc, dma_eng).dma_start(t_[:], xvb_d[:, s])
                    xvt[s] = t_

                kT8 = {}
                vaug = {}     # per kv pair: [128, 2, 65] fp8
                vaug0b = None  # pair 0 in bf16

                def emit_k_super(s):
                    ps0 = projp.tile([128, SUP], F32, tag="pa")
                    ps1 = projp.tile([128, SUP], F32, tag="pb")
                    # K projection (fp8 DR): plane hj -> ps<hj>[32:64]
                    for hj, pst in ((0, ps0), (1, ps1)):
                        for i in range(2):
                            for c2 in range(4):
                                nc.tensor.matmul(
                                    pst[0:32, 256 * i:256 * (i + 1)],
                                    wk8_sb[:, c2, :, hj, :],
                                    x8t[s][:, :, i, c2, 0:256],
                                    start=(c2 == 0), stop=(c2 == 3),
                                    perf_mode=DR)
                    kt = kTp.tile([32, 2, SUP], FP8, tag=f"kT_{s}")
                    nc.vector.tensor_scalar_add(
                        kt[:, 0, :], ps0[0:32, :], bias_sb[0:32, 2:3])
                    nc.vector.tensor_scalar_add(
                        kt[:, 1, :], ps1[0:32, :], bias_sb[0:32, 3:4])
                    kT8[s] = kt

                def emit_v_super(s):
                    nonlocal vaug0b
                    psv = vxp.tile([128, SUP], F32, tag="vx")
                    for c in range(8):
                        nc.tensor.matmul(psv[64:128, :],
                                         wvb_sb[:, c, :],
                                         xvt[s][:, c, :],
                                         start=(c == 0), stop=(c == 7))
                    vt = vTp.tile([128, SUP], BF16, tag=f"vT_{s}")
                    nc.vector.tensor_scalar_add(
                        vt[64:128, :], psv[64:128, :], bias_sb[64:128, 0:1])
                    # vaug pairs: pair pp = 2s + half, chunks at cols
                    # 256*half + (0:128, 128:256)
                    for half in range(2):
                        tp = vxp.tile([128, 2, H + 2], BF16, tag="vx")
                        for m in range(2):
                            nc.tensor.transpose(
                                tp[:, m, 0:H],
                                vt[64:128, 256 * half + KC * m:
                                   256 * half + KC * (m + 1)],
                                idb_sb[64:128, 64:128])
                        va = vap.tile([128, 2, H + 16], FP8, tag=f"va{2*s+half}")
                        getattr(nc, memset_eng).memset(va[:, :, H:H + 1], 1.0)
                        nc.vector.tensor_copy(va[:, :, 0:H], tp[:, :, 0:H])
                        vaug[2 * s + half] = va
                        if s == 0 and half == 0:
                            vab = vap.tile([128, 2, H + 1], BF16, tag="va0b")
                            getattr(nc, memset_eng).memset(vab[:, :, H:H + 1], 1.0)
                            nc.vector.tensor_copy(vab[:, :, 0:H], tp[:, :, 0:H])
                            vaug0b = vab

                pts = {}

                def emit_att_qs(sg):
                    q0 = projp.tile([128, SUP], F32, tag="pa")
                    q1 = projp.tile([128, SUP], F32, tag="pb")
                    pr, i = sg // 2, sg % 2
                    for hj, pst in ((0, q0), (1, q1)):
                        for c2 in range(4):
                            nc.tensor.matmul(
                                pst[0:32, :],
                                wq8_sb[:, c2, :, hj, :],
                                x8t[pr][:, :, i, c2, :],
                                start=(c2 == 0), stop=(c2 == 3),
                                perf_mode=DR)
                    qt = qTp.tile([32, 2, SUP], FP8, tag=f"qT_{sg}")
                    nc.vector.tensor_scalar_add(
                        qt[:, 0, :], q0[0:32, :], bias_sb[0:32, 0:1])
                    nc.vector.tensor_scalar_add(
                        qt[:, 1, :], q1[0:32, :], bias_sb[0:32, 1:2])

                    bf = (sg == 0)
                    for kp in range(sg + 1):
                        skv, base = kp // 2, 256 * (kp % 2)
                        sps = spp.tile([128, 2, SUP], F32)
                        for m in range(2):
                            nc.tensor.matmul(
                                sps[:, m, :],
                                kT8[skv][:, :, base + KC * m:base + KC * (m + 1)],
                                qt[:, :, :],
                                start=True, stop=True, perf_mode=DR)
                        pt = ptp.tile([128, 2, SUP], BF16 if bf else FP8)
                        nc.scalar.activation(pt[:], sps[:],
                                             mybir.ActivationFunctionType.Exp,
                                             scale=scale)
                        if kp == sg:
                            if bf:
                                getattr(nc, mask_eng).tensor_mul(pt[:], pt[:], maskb_sb[:])
                            else:
                                getattr(nc, mask_eng).tensor_mul(pt[:], pt[:], mask8_sb[:])
                        pts[sg, kp] = pt

                def emit_att_pv(sg):
                    bf = (sg == 0)
                    ot_ps = otp.tile([H + 1, SUP], F32)
                    for kp in range(sg + 1):
                        pt = pts.pop((sg, kp))
                        if bf:
                            for m in range(2):
                                nc.tensor.matmul(
                                    ot_ps[:], vaug0b[:, m, :], pt[:, m, :],
                                    start=(m == 0), stop=(m == 1))
                        else:
                            va = vaug[kp]
                            nc.tensor.matmul(
                                ot_ps[:], va[:, :, 0:H + 1], pt[:],
                                start=(kp == 0), stop=(kp == sg),
                                perf_mode=DR)
                    ot_s = otsb.tile([H + 1, SUP], BF16)
                    nc.vector.tensor_copy(ot_s[:], ot_ps[:])
                    nc.sync.dma_start(out_d[rep, sg], ot_s[:])

                # all input DMA issues up front (x8-heavy first)
                load_x8(0)
                load_x8(1)
                load_xv(0)
                load_x8(2)
                load_xv(1)
                load_x8(3)
                load_xv(2)
                load_xv(3)
                # one-super lag between qs (exp) and pv so PE never waits ACT
                emits = {"k": emit_k_super, "v": emit_v_super,
                         "q": emit_att_qs, "p": emit_att_pv}
                for step in order:
                    emits[step[0]](int(step[1]))
    nc.compile()
    return nc


def make_core_inputs(xT_b, wq8, wk8, wvb, bq64, bk64, bv, h, T):
    """Per-core inputs. xT_b: [C, T] f32 for this core's batch."""
    n_sup = T // SUP
    n_pair = n_sup // 2
    perm = chunk_perm(h)

    # q-column permutation (parity chunks first within each 512-super)
    xp = xT_b.reshape(C, n_sup, 4, KC)[:, :, perm, :].reshape(C, T)
    x8 = xp.astype(E4NP)
    # [p, pr, jc, i, cc2, t]: row c = cc2*256 + jc*128 + p,
    # col = (2pr+i)*512 + t
    xq8 = np.ascontiguousarray(
        x8.reshape(4, 2, 128, n_pair, 2, SUP).transpose(2, 3, 1, 4, 0, 5))

    # parity kv columns in local order (no permutation)
    xv = xT_b.reshape(C, T // 256, 2, KC)[:, :, h, :].reshape(C, T // 2)
    xvb = np.ascontiguousarray(
        xv.astype(BFNP).reshape(8, 128, n_pair, SUP).transpose(1, 2, 0, 3))

    mask = np.zeros((128, 2, SUP), dtype=np.float32)
    p = np.arange(128)[:, None]
    col = np.arange(SUP)[None, :]
    qrel = np.asarray(perm)[col // KC] * KC + col % KC
    for m in range(2):
        kvrel = (2 * m + h) * KC + p
        mask[:, m, :] = (kvrel <= qrel)

    return {
        "xq8": xq8,
        "xvb": xvb,
        "wq8": wq8, "wk8": wk8, "wvb": wvb,
        "bias": make_bias(bq64, bk64, bv),
        "mask8": mask.astype(E4NP),
        "maskb": mask.astype(BFNP),
        "idb": np.eye(128, dtype=BFNP),
    }


def make_bias(bq64, bk64, bv):
    bias = np.zeros((128, 4), np.float32)
    bias[0:32, 0] = bq64[0:32]
    bias[0:32, 1] = bq64[32:64]
    bias[0:32, 2] = bk64[0:32]
    bias[0:32, 3] = bk64[32:64]
    bias[64:128, 0] = bv
    return bias


def pack_w8(W):
    """[C, 64] -> [128, 4(cc2), 2(jc), 2(hj), 32] fp8 of 64*W."""
    w = (64.0 * np.asarray(W, np.float32)).astype(E4NP)
    return np.ascontiguousarray(
        w.reshape(4, 2, 128, 2, 32).transpose(2, 0, 1, 3, 4))


def prep_inputs(x, Wq, bq, Wk, bk, Wv, bv, T):
    x = np.asarray(x, np.float32)
    wq8 = pack_w8(Wq)
    wk8 = pack_w8(Wk)
    wvb = np.ascontiguousarray(
        np.asarray(Wv, np.float32).astype(BFNP).reshape(8, 128, H)
        .transpose(1, 0, 2))
    bq64 = 64.0 * np.asarray(bq, np.float32)
    bk64 = 64.0 * np.asarray(bk, np.float32)
    bvf = np.asarray(bv, np.float32)
    n_b = x.shape[0]
    out = []
    for core in range(2 * n_b):
        b, h = core // 2, core % 2
        xT_b = np.ascontiguousarray(x[b].T)
        out.append(make_core_inputs(xT_b, wq8, wk8, wvb, bq64, bk64, bvf, h, T))
    return out


def unpermute_rows(arr, h, T):
    n_sup = T // SUP
    perm = np.asarray(chunk_perm(h))
    a = arr.reshape(n_sup, 4, KC, -1)
    out = np.empty_like(a)
    out[:, perm, :, :] = a
    return out.reshape(T, -1)


def combine(results, T):
    n_b = len(results) // 2
    out = np.empty((n_b, T, H), np.float32)
    for b in range(n_b):
        parts = []
        for h in range(2):
            o = np.asarray(results[2 * b + h]["out"], np.float32)
            o = o.reshape(-1, H + 1, SUP)[:T // SUP]       # rep 0
            o = o.transpose(0, 2, 1).reshape(T, H + 1)     # [T, 65] permuted
            parts.append(unpermute_rows(o, h, T).astype(np.float64))
        num = parts[0][:, :H] + parts[1][:, :H]
        den = parts[0][:, H:] + parts[1][:, H:]
        out[b] = (num / den).astype(np.float32)
    return out


_NC = None


def kernel(x, Wq, bq, Wk, bk, Wv, bv):
    global _NC
    T = np.asarray(x).shape[1]
    if _NC is None:
        _NC = build_nc(T)
    in_maps = prep_inputs(x, Wq, bq, Wk, bk, Wv, bv, T)
    res = run_bass_kernel_spmd(_NC, in_maps, core_ids=list(range(8)))
    return combine(res.results, T)
